# revision 50
# baseline (speedup 1.0000x reference)
"""AdaptiveTripletLoss on 8 TRN2 NeuronCores (Bass/Tile, SPMD).

Math: loss = mean over valid rows of relu(hp - hn + 0.5*(1+hp)) with
  hp = hardest (max) same-label distance, hn = hardest (min) other-label
  distance, distances on L2-normalized embeddings.

Device algorithm (per core, rows sharded):
  G' = En @ En.T - 8 * H @ H.T   (H = one-hot labels, 128 classes = full K)
  Same-label entries of G' sit in [-9,-7], different-label in [-1,1], so
    hn^2 = relu(2 - 2*max_j G')          (max over ALL columns)
    hp^2 = relu(-14 - 2*min_j G')        (min over <=2 static 512-chunks)
  Rows are sorted by label on the host and each core's column copy is
  rotated so its own 1024 rows sit at local columns [512, 1536); the
  same-label columns of a 128-row tile then fall in at most two static
  512-chunks, so the -8*HH' correction and the min-reduce only touch
  those chunks.

  Matmuls run in bf16 (fp32 PSUM accumulation; verified rel err ~1e-5).
  The full-row max scan is split between VectorE (tensor_reduce max) and
  ScalarE (exp + free accum_out => log-sum-exp max, beta=160), keeping
  both engines busy; GpSimd computes the squares for row norms.
"""

import sys

sys.path.insert(0, "/opt/trn_rl_repo")

import numpy as np

N_CORES = 8
B, D, NCLS = 8192, 128, 128
BC = B // N_CORES          # rows per core
ROLL = 512                 # own rows at local columns [ROLL, ROLL+BC)
NQ, QW = 4, 2048           # ET quarters
M_TILES = BC // 128        # 8 row tiles per core
G_GROUPS = 8               # 8 column groups of 1024
HTW_W = 2048               # one-hot window: local chunks 0..3
ACT_GROUPS = (4, 5, 6, 7)  # max-scan groups handled by ScalarE LSE
BETA = 160.0               # LSE sharpness; bias point keeps exp in range
LSE_BIAS = 0.6
MC = 96                    # max class size bound (asserted at prep time)

_cache = {}


def _build():
    import concourse.tile as tile
    from concourse import bacc, mybir

    f32 = mybir.dt.float32
    bf16 = mybir.dt.bfloat16
    AX = mybir.AxisListType
    OP = mybir.AluOpType
    AF = mybir.ActivationFunctionType
    from concourse.bass import MemorySpace

    n_dve = G_GROUPS - len(ACT_GROUPS)
    dve_groups = [g for g in range(G_GROUPS) if g not in ACT_GROUPS]

    nc = bacc.Bacc("TRN2", target_bir_lowering=False, debug=False,
                   num_devices=N_CORES)
    emb_ext = nc.dram_tensor("emb", [128, B], bf16, kind="ExternalInput")
    htw_ext = nc.dram_tensor("htw", [NCLS, HTW_W], bf16, kind="ExternalInput")
    htwn_ext = nc.dram_tensor("htwn", [NCLS, BC], bf16, kind="ExternalInput")
    ident_ext = nc.dram_tensor("ident", [128, 128], bf16, kind="ExternalInput")
    out_ext = nc.dram_tensor("out", [1, 2], f32, kind="ExternalOutput")

    with tile.TileContext(nc) as tc:
        with (
            tc.tile_pool(name="persist", bufs=1) as pp,
            tc.tile_pool(name="sq", bufs=2) as sq_pool,
            tc.tile_pool(name="nen", bufs=2) as nen_pool,
            tc.tile_pool(name="hneg", bufs=2) as hneg_pool,
            tc.tile_pool(name="fin", bufs=1) as fin_pool,
            tc.tile_pool(name="tp_ps", bufs=2, space=MemorySpace.PSUM) as tp_ps,
            tc.tile_pool(name="mm_ps", bufs=3, space=MemorySpace.PSUM) as mm_ps,
        ):
            # ---------- persistent SBUF ----------
            enat = [pp.tile([128, QW // 2], bf16, name=f"enat{qh}",
                             tag=f"enat{qh}") for qh in range(NQ * 2)]
            etq = [pp.tile([128, QW], bf16, name=f"et{q}", tag=f"et{q}")
                   for q in range(NQ)]
            htw = pp.tile([NCLS, HTW_W], bf16, name="htw_sb", tag="htw_sb")
            ident = pp.tile([128, 128], bf16, name="ident_sb", tag="ident_sb")
            ssq = pp.tile([128, 64], f32, name="ssq", tag="ssq")
            rvec = pp.tile([128, 64], f32, name="rvec", tag="rvec")
            gmaxf = pp.tile([128, M_TILES * G_GROUPS], f32, name="gmaxf",
                            tag="gmaxf")
            gsum = pp.tile([128, M_TILES * G_GROUPS], f32, name="gsum",
                           tag="gsum")
            gminw = pp.tile([128, M_TILES * 2], f32, name="gminw", tag="gminw")
            bexp = pp.tile([128, 1], f32, name="bexp", tag="bexp")

            nc.gpsimd.memset(bexp[:], -float(BETA * LSE_BIAS))
            nc.gpsimd.memset(gminw[:], 1e9)
            nc.gpsimd.memset(gmaxf[:], -1e9)
            nc.gpsimd.memset(gsum[:], 0.0)
            nc.sync.dma_start(ident[:], ident_ext.ap())
            nc.sync.dma_start(htw[:], htw_ext.ap())
            htwn = pp.tile([NCLS, BC], bf16, name="htwn_sb", tag="htwn_sb")
            nc.sync.dma_start(htwn[:], htwn_ext.ap())

            # ---------- prologue (per quarter, pipelined):
            # load -> ssq -> r -> normalize -> transpose ----------
            emb_ap = emb_ext.ap()
            i32 = mybir.dt.int32
            HW = QW // 2  # half-quarter: 8 row tiles = 1024 rows
            for qh in range(NQ * 2):
                # host pre-tiles emb to [p, t*128+d]; contiguous per partition
                nc.sync.dma_start(
                    enat[qh][:],
                    emb_ap[:, qh * 1024:(qh + 1) * 1024])
            # -8 * one-hot of own rows comes prescaled from the host
            hnegs = [htwn[:, m * 128:(m + 1) * 128] for m in range(M_TILES)]

            def build_quarter(q):
                for h in range(2):
                    qh = q * 2 + h
                    hs = h * HW
                    o = q * 16 + h * 8
                    qs = ssq[:, o:o + 8]
                    if qh < 5:
                        sq = sq_pool.tile([128, HW], f32)
                        nc.gpsimd.tensor_tensor(sq[:], enat[qh][:],
                                                enat[qh][:],
                                                op=OP.mult)
                        nc.vector.tensor_reduce(
                            qs, sq[:].rearrange("p (t d) -> p t d", d=128),
                            axis=AX.X, op=OP.add)
                    else:
                        # ScalarE square with free accumulation per tile
                        for t in range(8):
                            sqt = sq_pool.tile([128, 128], bf16, tag="sqt",
                                               bufs=2, name="sqt")
                            nc.scalar.activation(
                                sqt[:], enat[qh][:, t * 128:(t + 1) * 128],
                                AF.Square, accum_out=qs[:, t:t + 1])
                    # r = rsqrt(ssq) on DVE only: int magic + 2 Newton steps
                    rq = rvec[:, o:o + 8]
                    yi = fin_pool.tile([128, 8], i32, tag="yi", bufs=2)
                    nc.vector.tensor_scalar(yi[:], qs.bitcast(i32), 1, None,
                                            op0=OP.arith_shift_right)
                    nc.vector.tensor_scalar(yi[:], yi[:], -1, 0x5F3759DF,
                                            op0=OP.mult, op1=OP.add)
                    y = yi[:].bitcast(f32)
                    w = fin_pool.tile([128, 8], f32, tag="w", bufs=2)
                    for _ in range(2):
                        nc.vector.tensor_tensor(w[:], y, y, op=OP.mult)
                        nc.vector.tensor_tensor(w[:], w[:], qs, op=OP.mult)
                        nc.vector.tensor_scalar(w[:], w[:], -0.5, 1.5,
                                                op0=OP.mult, op1=OP.add)
                        nc.vector.tensor_tensor(rq, y, w[:], op=OP.mult)
                        y = rq

                    # batched diag build: diag[p, t, j] = ident[p,j]*r[p,t]
                    diag = nen_pool.tile([128, HW], bf16)
                    rb = rq.rearrange("p (t o) -> p t o", o=1) \
                        .broadcast_to((128, 8, 128))
                    ib = ident[:].rearrange("p (o j) -> p o j", o=1) \
                        .broadcast_to((128, 8, 128))
                    nc.vector.tensor_tensor(
                        diag[:].rearrange("p (t d) -> p t d", d=128),
                        ib, rb, op=OP.mult)

                    # normalize+transpose in one matmul per tile:
                    # out[d,j] = sum_k E[k,d]*r_k*delta_kj = E[j,d]*r_j
                    for t4 in range(2):
                        tp = tp_ps.tile([128, 512], f32, tag="tp")
                        for tt in range(4):
                            t = t4 * 4 + tt
                            nc.tensor.matmul(
                                tp[:, tt * 128:(tt + 1) * 128],
                                enat[qh][:, t * 128:(t + 1) * 128],
                                diag[:, t * 128:(t + 1) * 128],
                                start=True, stop=True)
                        nc.scalar.copy(
                            etq[q][:, hs + t4 * 512:hs + (t4 + 1) * 512],
                            tp[:])

            # ---------- main: G' tiles + row reduces (group-outer) ----------
            def main_group(g, m):
                c0 = ROLL + m * 128
                win_chunks = sorted({(c0 - MC + 1) // 512,
                                     (c0 + 127 + MC) // 512})
                kxm_e = etq[0][:, c0:c0 + 128]
                act_ms = (0, 1, 3, 5, 6) if g < 4 else (2, 4, 7)
                use_act = m in act_ms
                ps = mm_ps.tile([128, 1024], f32, tag="mmg", name="ps")
                qq, qoff = g // 2, (g % 2) * 1024
                # E matmuls at PSUM-bank (512) granularity; windowed
                # chunks get the -8*HH' accumulation on top
                for cc in range(2):
                    a, b = cc * 512, (cc + 1) * 512
                    winpart = (2 * g + cc) in win_chunks
                    nc.tensor.matmul(
                        ps[:, a:b], kxm_e,
                        etq[qq][:, qoff + a:qoff + b],
                        start=True, stop=not winpart)
                for cc in range(2):
                    ch = 2 * g + cc
                    if ch in win_chunks:
                        a, b = cc * 512, (cc + 1) * 512
                        nc.tensor.matmul(
                            ps[:, a:b], hnegs[m],
                            htw[:, ch * 512:(ch + 1) * 512],
                            start=False, stop=True)
                        # pos side: min of G' over this window chunk
                        k = m * 2 + win_chunks.index(ch)
                        nc.vector.tensor_reduce(
                            gminw[:, k:k + 1],
                            ps[:, a:b], axis=AX.X, op=OP.min)
                # neg side: max of G' over everything; split DVE/ACT so ACT
                # gets more groups early (it idles during the prologue)
                k = m * G_GROUPS + g
                if use_act:
                    nc.scalar.activation(ps[:], ps[:], AF.Exp,
                                         scale=float(BETA), bias=bexp[:],
                                         accum_out=gsum[:, k:k + 1])
                else:
                    nc.vector.tensor_reduce(
                        gmaxf[:, k:k + 1], ps[:], axis=AX.X, op=OP.max)

            for q in range(NQ):
                build_quarter(q)
                for g in (2 * q, 2 * q + 1):
                    for m in range(M_TILES):
                        main_group(g, m)

            # ---------- finalize (all small ops; DVE + bit tricks) ----------
            # direct part of the max scan
            gmaxd = fin_pool.tile([128, M_TILES], f32, tag="gmaxd")
            nc.vector.tensor_reduce(
                gmaxd[:], gmaxf[:].rearrange("p (m g) -> p m g", g=G_GROUPS),
                axis=AX.X, op=OP.max)
            # LSE part: max ~= LSE_BIAS + ln(sum S)/beta, with
            # ln(x) ~= (float(bits(x)) * 2^-23 - 126.9427) * ln2
            ssum = fin_pool.tile([128, M_TILES], f32, tag="ssum")
            nc.vector.tensor_reduce(
                ssum[:], gsum[:].rearrange("p (m g) -> p m g", g=G_GROUPS),
                axis=AX.X, op=OP.add)
            ssumf = fin_pool.tile([128, M_TILES], f32, tag="ssumf")
            nc.vector.tensor_scalar_max(ssumf[:], ssum[:], 1e-37)
            ssbits = fin_pool.tile([128, M_TILES], f32, tag="ssbits")
            nc.vector.tensor_copy(ssbits[:], ssumf[:].bitcast(i32))
            LN2 = 0.69314718056
            lsemax = fin_pool.tile([128, M_TILES], f32, tag="lsemax")
            nc.vector.tensor_scalar(
                lsemax[:], ssbits[:], LN2 / (BETA * 8388608.0),
                LSE_BIAS - 126.94269504 * LN2 / BETA, op0=OP.mult, op1=OP.add)
            gmax8 = fin_pool.tile([128, M_TILES], f32, tag="gmax8")
            nc.vector.tensor_tensor(gmax8[:], gmaxd[:], lsemax[:], op=OP.max)

            gmin8 = fin_pool.tile([128, M_TILES], f32, tag="gmin8")
            nc.vector.tensor_reduce(
                gmin8[:], gminw[:].rearrange("p (m k) -> p m k", k=2),
                axis=AX.X, op=OP.min)

            # hn = sqrt(relu(2-2*gmax8)), hp = sqrt(relu(-14-2*gmin8));
            # sqrt(x) = x * rsqrt(x) with magic + 2 Newton steps (no ACT)
            def dve_sqrt(name, src_ap, scale, bias):
                x = fin_pool.tile([128, M_TILES], f32, tag=f"{name}_x")
                nc.vector.tensor_scalar(x[:], src_ap, scale, bias,
                                        op0=OP.mult, op1=OP.add)
                xm = fin_pool.tile([128, M_TILES], f32, tag=f"{name}_xm")
                nc.vector.tensor_scalar_max(xm[:], x[:], 1e-20)
                yi = fin_pool.tile([128, M_TILES], i32, tag=f"{name}_yi")
                nc.vector.tensor_scalar(yi[:], xm[:].bitcast(i32), 1, None,
                                        op0=OP.arith_shift_right)
                nc.vector.tensor_scalar(yi[:], yi[:], -1, 0x5F3759DF,
                                        op0=OP.mult, op1=OP.add)
                y = yi[:].bitcast(f32)
                w = fin_pool.tile([128, M_TILES], f32, tag=f"{name}_w")
                for _ in range(2):
                    nc.vector.tensor_tensor(w[:], y, y, op=OP.mult)
                    nc.vector.tensor_tensor(w[:], w[:], xm[:], op=OP.mult)
                    nc.vector.tensor_scalar(w[:], w[:], -0.5, 1.5,
                                            op0=OP.mult, op1=OP.add)
                    nc.vector.tensor_tensor(y, y, w[:], op=OP.mult)
                out = fin_pool.tile([128, M_TILES], f32, tag=f"{name}")
                nc.vector.tensor_tensor(out[:], y, xm[:], op=OP.mult)
                return out

            hn = dve_sqrt("hn", gmax8[:], -2.0, 2.0)
            hp = dve_sqrt("hp", gmin8[:], -2.0, -14.0)

            # every row is valid (asserted at prep: all classes >=2 members,
            # >1 class present), so just sum the per-row losses
            t1 = fin_pool.tile([128, M_TILES], f32, tag="t1")
            nc.vector.tensor_scalar(t1[:], hp[:], 1.5, 0.5,
                                    op0=OP.mult, op1=OP.add)
            t2 = fin_pool.tile([128, M_TILES], f32, tag="t2")
            nc.vector.tensor_tensor(t2[:], t1[:], hn[:], op=OP.subtract)
            t3 = fin_pool.tile([128, M_TILES], f32, tag="t3")
            nc.vector.tensor_scalar_max(t3[:], t2[:], 0.0)

            stacked = fin_pool.tile([128, 2], f32, tag="stacked")
            nc.vector.tensor_reduce(stacked[:, 0:1], t3[:], axis=AX.X, op=OP.add)
            nc.gpsimd.memset(stacked[:, 1:2], float(M_TILES))

            ones = fin_pool.tile([128, 1], f32, tag="ones")
            nc.gpsimd.memset(ones[:], 1.0)
            pfin = tp_ps.tile([1, 2], f32, tag="tp")
            nc.tensor.matmul(pfin[:], ones[:], stacked[:], start=True, stop=True)
            outsb = fin_pool.tile([1, 2], f32, tag="outsb")
            nc.vector.tensor_copy(outsb[:], pfin[:])
            nc.sync.dma_start(out_ext.ap(), outsb[:])

    nc.compile()
    return nc


def _get_nc():
    if "nc" not in _cache:
        _cache["nc"] = _build()
    return _cache["nc"]


def _prep_inputs(embeddings, labels):
    import ml_dtypes

    emb = np.ascontiguousarray(np.asarray(embeddings, dtype=np.float32))
    lab = np.asarray(labels).astype(np.int64).ravel()
    assert emb.shape == (B, D) and lab.shape == (B,)

    counts = np.bincount(lab, minlength=NCLS)
    present = counts[counts > 0]
    # window scheme needs bounded class extent; singleton classes would
    # change validity semantics. Both hold for this problem's data.
    assert present.max() <= MC, f"class too large for window: {present.max()}"
    assert present.min() >= 2, "singleton class unsupported"

    perm = np.argsort(lab, kind="stable")
    emb_s = emb[perm].astype(ml_dtypes.bfloat16)
    lab_s = lab[perm]

    ident = np.eye(128, dtype=ml_dtypes.bfloat16)
    in_maps = []
    for c in range(N_CORES):
        shift = ROLL - BC * c
        emb_l = np.roll(emb_s, shift, axis=0)
        lab_l = np.roll(lab_s, shift)
        htw = (lab_l[None, :HTW_W] == np.arange(NCLS)[:, None]).astype(
            ml_dtypes.bfloat16)
        # pre-tile to partition-major [p, t*128+d] so DMA reads contiguously
        emb_t = emb_l.reshape(64, 128, 128).transpose(1, 0, 2).reshape(128, B)
        htwn = (-8.0 * htw[:, ROLL:ROLL + BC].astype(np.float32)).astype(
            ml_dtypes.bfloat16)
        in_maps.append({
            "emb": np.ascontiguousarray(emb_t),
            "htw": np.ascontiguousarray(htw),
            "htwn": np.ascontiguousarray(htwn),
            "ident": ident,
        })
    return in_maps


def kernel(embeddings, labels, _trace=False):
    from concourse.bass_utils import run_bass_kernel_spmd

    nc = _get_nc()
    in_maps = _prep_inputs(embeddings, labels)
    res = run_bass_kernel_spmd(nc, in_maps, core_ids=list(range(N_CORES)),
                               trace=_trace)
    total = 0.0
    count = 0.0
    for c in range(N_CORES):
        o = np.asarray(res.results[c]["out"], dtype=np.float64)
        total += o[0, 0]
        count += o[0, 1]
    if _trace:
        _cache["last_exec_time_ns"] = res.exec_time_ns
        _cache["last_results"] = res
    return np.float32(total / max(count, 1.0))


# revision 51
# speedup vs baseline: 1.0161x; 1.0161x over previous
"""AdaptiveTripletLoss on 8 TRN2 NeuronCores (Bass/Tile, SPMD).

Math: loss = mean over valid rows of relu(hp - hn + 0.5*(1+hp)) with
  hp = hardest (max) same-label distance, hn = hardest (min) other-label
  distance, distances on L2-normalized embeddings.

Device algorithm (per core, rows sharded):
  G' = En @ En.T - 8 * H @ H.T   (H = one-hot labels, 128 classes = full K)
  Same-label entries of G' sit in [-9,-7], different-label in [-1,1], so
    hn^2 = relu(2 - 2*max_j G')          (max over ALL columns)
    hp^2 = relu(-14 - 2*min_j G')        (min over <=2 static 512-chunks)
  Rows are sorted by label on the host and each core's column copy is
  rotated so its own 1024 rows sit at local columns [512, 1536); the
  same-label columns of a 128-row tile then fall in at most two static
  512-chunks, so the -8*HH' correction and the min-reduce only touch
  those chunks.

  Matmuls run in bf16 (fp32 PSUM accumulation; verified rel err ~1e-5).
  The full-row max scan is split between VectorE (tensor_reduce max) and
  ScalarE (exp + free accum_out => log-sum-exp max, beta=160), keeping
  both engines busy; GpSimd computes the squares for row norms.
"""

import sys

sys.path.insert(0, "/opt/trn_rl_repo")

import numpy as np

N_CORES = 8
B, D, NCLS = 8192, 128, 128
BC = B // N_CORES          # rows per core
ROLL = 512                 # own rows at local columns [ROLL, ROLL+BC)
NQ, QW = 4, 2048           # ET quarters
M_TILES = BC // 128        # 8 row tiles per core
G_GROUPS = 8               # 8 column groups of 1024
HTW_W = 2048               # one-hot window: local chunks 0..3
ACT_GROUPS = (4, 5, 6, 7)  # max-scan groups handled by ScalarE LSE
BETA = 160.0               # LSE sharpness; bias point keeps exp in range
LSE_BIAS = 0.6
MC = 96                    # max class size bound (asserted at prep time)

_cache = {}


def _build():
    import concourse.tile as tile
    from concourse import bacc, mybir

    f32 = mybir.dt.float32
    bf16 = mybir.dt.bfloat16
    AX = mybir.AxisListType
    OP = mybir.AluOpType
    AF = mybir.ActivationFunctionType
    from concourse.bass import MemorySpace

    n_dve = G_GROUPS - len(ACT_GROUPS)
    dve_groups = [g for g in range(G_GROUPS) if g not in ACT_GROUPS]

    nc = bacc.Bacc("TRN2", target_bir_lowering=False, debug=False,
                   num_devices=N_CORES)
    emb_ext = nc.dram_tensor("emb", [128, B], bf16, kind="ExternalInput")
    htw_ext = nc.dram_tensor("htw", [NCLS, HTW_W], bf16, kind="ExternalInput")
    htwn_ext = nc.dram_tensor("htwn", [NCLS, BC], bf16, kind="ExternalInput")
    ident_ext = nc.dram_tensor("ident", [128, 128], bf16, kind="ExternalInput")
    out_ext = nc.dram_tensor("out", [1, 2], f32, kind="ExternalOutput")

    with tile.TileContext(nc) as tc:
        with (
            tc.tile_pool(name="persist", bufs=1) as pp,
            tc.tile_pool(name="sq", bufs=2) as sq_pool,
            tc.tile_pool(name="nen", bufs=2) as nen_pool,
            tc.tile_pool(name="hneg", bufs=2) as hneg_pool,
            tc.tile_pool(name="fin", bufs=1) as fin_pool,
            tc.tile_pool(name="tp_ps", bufs=2, space=MemorySpace.PSUM) as tp_ps,
            tc.tile_pool(name="mm_ps", bufs=3, space=MemorySpace.PSUM) as mm_ps,
        ):
            # ---------- persistent SBUF ----------
            enat = [pp.tile([128, QW // 2], bf16, name=f"enat{qh}",
                             tag=f"enat{qh}") for qh in range(NQ * 2)]
            etq = [pp.tile([128, QW], bf16, name=f"et{q}", tag=f"et{q}")
                   for q in range(NQ)]
            htw = pp.tile([NCLS, HTW_W], bf16, name="htw_sb", tag="htw_sb")
            ident = pp.tile([128, 128], bf16, name="ident_sb", tag="ident_sb")
            ssq = pp.tile([128, 64], f32, name="ssq", tag="ssq")
            rvec = pp.tile([128, 64], f32, name="rvec", tag="rvec")
            gmaxf = pp.tile([128, M_TILES * G_GROUPS], f32, name="gmaxf",
                            tag="gmaxf")
            gsum = pp.tile([128, M_TILES * G_GROUPS], f32, name="gsum",
                           tag="gsum")
            gminw = pp.tile([128, M_TILES * 2], f32, name="gminw", tag="gminw")
            bexp = pp.tile([128, 1], f32, name="bexp", tag="bexp")

            nc.gpsimd.memset(bexp[:], -float(BETA * LSE_BIAS))
            nc.gpsimd.memset(gminw[:], 1e9)
            nc.gpsimd.memset(gmaxf[:], -1e9)
            nc.gpsimd.memset(gsum[:], 0.0)
            nc.sync.dma_start(ident[:], ident_ext.ap())
            nc.sync.dma_start(htw[:], htw_ext.ap())
            htwn = pp.tile([NCLS, BC], bf16, name="htwn_sb", tag="htwn_sb")
            nc.sync.dma_start(htwn[:], htwn_ext.ap())

            # ---------- prologue (per quarter, pipelined):
            # load -> ssq -> r -> normalize -> transpose ----------
            emb_ap = emb_ext.ap()
            i32 = mybir.dt.int32
            HW = QW // 2  # half-quarter: 8 row tiles = 1024 rows
            for qh in range(NQ * 2):
                # host pre-tiles emb to [p, t*128+d]; contiguous per partition
                nc.sync.dma_start(
                    enat[qh][:],
                    emb_ap[:, qh * 1024:(qh + 1) * 1024])
            # -8 * one-hot of own rows comes prescaled from the host
            hnegs = [htwn[:, m * 128:(m + 1) * 128] for m in range(M_TILES)]

            def build_quarter(q):
                for h in range(2):
                    qh = q * 2 + h
                    hs = h * HW
                    sq = sq_pool.tile([128, HW], f32)
                    nc.gpsimd.tensor_tensor(sq[:], enat[qh][:],
                                            enat[qh][:],
                                            op=OP.mult)
                    o = q * 16 + h * 8
                    qs = ssq[:, o:o + 8]
                    nc.vector.tensor_reduce(
                        qs, sq[:].rearrange("p (t d) -> p t d", d=128),
                        axis=AX.X, op=OP.add)
                    # r = rsqrt(ssq) on DVE only: int magic + 2 Newton steps
                    rq = rvec[:, o:o + 8]
                    yi = fin_pool.tile([128, 8], i32, tag="yi", bufs=2)
                    nc.vector.tensor_scalar(yi[:], qs.bitcast(i32), 1, None,
                                            op0=OP.arith_shift_right)
                    nc.vector.tensor_scalar(yi[:], yi[:], -1, 0x5F3759DF,
                                            op0=OP.mult, op1=OP.add)
                    y = yi[:].bitcast(f32)
                    w = fin_pool.tile([128, 8], f32, tag="w", bufs=2)
                    for _ in range(2):
                        nc.vector.tensor_tensor(w[:], y, y, op=OP.mult)
                        nc.vector.tensor_tensor(w[:], w[:], qs, op=OP.mult)
                        nc.vector.tensor_scalar(w[:], w[:], -0.5, 1.5,
                                                op0=OP.mult, op1=OP.add)
                        nc.vector.tensor_tensor(rq, y, w[:], op=OP.mult)
                        y = rq

                    # batched diag build: diag[p, t, j] = ident[p,j]*r[p,t]
                    diag = nen_pool.tile([128, HW], bf16)
                    rb = rq.rearrange("p (t o) -> p t o", o=1) \
                        .broadcast_to((128, 8, 128))
                    ib = ident[:].rearrange("p (o j) -> p o j", o=1) \
                        .broadcast_to((128, 8, 128))
                    nc.vector.tensor_tensor(
                        diag[:].rearrange("p (t d) -> p t d", d=128),
                        ib, rb, op=OP.mult)

                    # normalize+transpose in one matmul per tile:
                    # out[d,j] = sum_k E[k,d]*r_k*delta_kj = E[j,d]*r_j
                    for t4 in range(2):
                        tp = tp_ps.tile([128, 512], f32, tag="tp")
                        for tt in range(4):
                            t = t4 * 4 + tt
                            nc.tensor.matmul(
                                tp[:, tt * 128:(tt + 1) * 128],
                                enat[qh][:, t * 128:(t + 1) * 128],
                                diag[:, t * 128:(t + 1) * 128],
                                start=True, stop=True)
                        nc.scalar.copy(
                            etq[q][:, hs + t4 * 512:hs + (t4 + 1) * 512],
                            tp[:])

            # ---------- main: G' tiles + row reduces (group-outer) ----------
            def main_group(g, m):
                c0 = ROLL + m * 128
                win_chunks = sorted({(c0 - MC + 1) // 512,
                                     (c0 + 127 + MC) // 512})
                kxm_e = etq[0][:, c0:c0 + 128]
                act_ms = (0, 1, 3, 5, 6) if g < 4 else (2, 4, 7)
                use_act = m in act_ms
                ps = mm_ps.tile([128, 1024], f32, tag="mmg", name="ps")
                qq, qoff = g // 2, (g % 2) * 1024
                # E matmuls at PSUM-bank (512) granularity; windowed
                # chunks get the -8*HH' accumulation on top
                for cc in range(2):
                    a, b = cc * 512, (cc + 1) * 512
                    winpart = (2 * g + cc) in win_chunks
                    nc.tensor.matmul(
                        ps[:, a:b], kxm_e,
                        etq[qq][:, qoff + a:qoff + b],
                        start=True, stop=not winpart)
                for cc in range(2):
                    ch = 2 * g + cc
                    if ch in win_chunks:
                        a, b = cc * 512, (cc + 1) * 512
                        nc.tensor.matmul(
                            ps[:, a:b], hnegs[m],
                            htw[:, ch * 512:(ch + 1) * 512],
                            start=False, stop=True)
                        # pos side: min of G' over this window chunk
                        k = m * 2 + win_chunks.index(ch)
                        nc.vector.tensor_reduce(
                            gminw[:, k:k + 1],
                            ps[:, a:b], axis=AX.X, op=OP.min)
                # neg side: max of G' over everything; split DVE/ACT so ACT
                # gets more groups early (it idles during the prologue)
                k = m * G_GROUPS + g
                if use_act:
                    nc.scalar.activation(ps[:], ps[:], AF.Exp,
                                         scale=float(BETA), bias=bexp[:],
                                         accum_out=gsum[:, k:k + 1])
                else:
                    nc.vector.tensor_reduce(
                        gmaxf[:, k:k + 1], ps[:], axis=AX.X, op=OP.max)

            for q in range(NQ):
                build_quarter(q)
                for g in (2 * q, 2 * q + 1):
                    for m in range(M_TILES):
                        main_group(g, m)

            # ---------- finalize (all small ops; DVE + bit tricks) ----------
            # direct part of the max scan
            gmaxd = fin_pool.tile([128, M_TILES], f32, tag="gmaxd")
            nc.vector.tensor_reduce(
                gmaxd[:], gmaxf[:].rearrange("p (m g) -> p m g", g=G_GROUPS),
                axis=AX.X, op=OP.max)
            # LSE part: max ~= LSE_BIAS + ln(sum S)/beta, with
            # ln(x) ~= (float(bits(x)) * 2^-23 - 126.9427) * ln2
            ssum = fin_pool.tile([128, M_TILES], f32, tag="ssum")
            nc.vector.tensor_reduce(
                ssum[:], gsum[:].rearrange("p (m g) -> p m g", g=G_GROUPS),
                axis=AX.X, op=OP.add)
            ssumf = fin_pool.tile([128, M_TILES], f32, tag="ssumf")
            nc.vector.tensor_scalar_max(ssumf[:], ssum[:], 1e-37)
            ssbits = fin_pool.tile([128, M_TILES], f32, tag="ssbits")
            nc.vector.tensor_copy(ssbits[:], ssumf[:].bitcast(i32))
            LN2 = 0.69314718056
            lsemax = fin_pool.tile([128, M_TILES], f32, tag="lsemax")
            nc.vector.tensor_scalar(
                lsemax[:], ssbits[:], LN2 / (BETA * 8388608.0),
                LSE_BIAS - 126.94269504 * LN2 / BETA, op0=OP.mult, op1=OP.add)
            gmax8 = fin_pool.tile([128, M_TILES], f32, tag="gmax8")
            nc.vector.tensor_tensor(gmax8[:], gmaxd[:], lsemax[:], op=OP.max)

            gmin8 = fin_pool.tile([128, M_TILES], f32, tag="gmin8")
            nc.vector.tensor_reduce(
                gmin8[:], gminw[:].rearrange("p (m k) -> p m k", k=2),
                axis=AX.X, op=OP.min)

            # hn = sqrt(relu(2-2*gmax8)), hp = sqrt(relu(-14-2*gmin8));
            # sqrt(x) = x * rsqrt(x) with magic + 2 Newton steps (no ACT)
            def dve_sqrt(name, src_ap, scale, bias):
                x = fin_pool.tile([128, M_TILES], f32, tag=f"{name}_x")
                nc.vector.tensor_scalar(x[:], src_ap, scale, bias,
                                        op0=OP.mult, op1=OP.add)
                xm = fin_pool.tile([128, M_TILES], f32, tag=f"{name}_xm")
                nc.vector.tensor_scalar_max(xm[:], x[:], 1e-20)
                yi = fin_pool.tile([128, M_TILES], i32, tag=f"{name}_yi")
                nc.vector.tensor_scalar(yi[:], xm[:].bitcast(i32), 1, None,
                                        op0=OP.arith_shift_right)
                nc.vector.tensor_scalar(yi[:], yi[:], -1, 0x5F3759DF,
                                        op0=OP.mult, op1=OP.add)
                y = yi[:].bitcast(f32)
                w = fin_pool.tile([128, M_TILES], f32, tag=f"{name}_w")
                for _ in range(2):
                    nc.vector.tensor_tensor(w[:], y, y, op=OP.mult)
                    nc.vector.tensor_tensor(w[:], w[:], xm[:], op=OP.mult)
                    nc.vector.tensor_scalar(w[:], w[:], -0.5, 1.5,
                                            op0=OP.mult, op1=OP.add)
                    nc.vector.tensor_tensor(y, y, w[:], op=OP.mult)
                out = fin_pool.tile([128, M_TILES], f32, tag=f"{name}")
                nc.vector.tensor_tensor(out[:], y, xm[:], op=OP.mult)
                return out

            hn = dve_sqrt("hn", gmax8[:], -2.0, 2.0)
            hp = dve_sqrt("hp", gmin8[:], -2.0, -14.0)

            # every row is valid (asserted at prep: all classes >=2 members,
            # >1 class present), so just sum the per-row losses
            t1 = fin_pool.tile([128, M_TILES], f32, tag="t1")
            nc.vector.tensor_scalar(t1[:], hp[:], 1.5, 0.5,
                                    op0=OP.mult, op1=OP.add)
            t2 = fin_pool.tile([128, M_TILES], f32, tag="t2")
            nc.vector.tensor_tensor(t2[:], t1[:], hn[:], op=OP.subtract)
            t3 = fin_pool.tile([128, M_TILES], f32, tag="t3")
            nc.vector.tensor_scalar_max(t3[:], t2[:], 0.0)

            stacked = fin_pool.tile([128, 2], f32, tag="stacked")
            nc.vector.tensor_reduce(stacked[:, 0:1], t3[:], axis=AX.X, op=OP.add)
            nc.gpsimd.memset(stacked[:, 1:2], float(M_TILES))

            ones = fin_pool.tile([128, 1], f32, tag="ones")
            nc.gpsimd.memset(ones[:], 1.0)
            pfin = tp_ps.tile([1, 2], f32, tag="tp")
            nc.tensor.matmul(pfin[:], ones[:], stacked[:], start=True, stop=True)
            outsb = fin_pool.tile([1, 2], f32, tag="outsb")
            nc.vector.tensor_copy(outsb[:], pfin[:])
            nc.sync.dma_start(out_ext.ap(), outsb[:])

    nc.compile()
    return nc


def _get_nc():
    if "nc" not in _cache:
        _cache["nc"] = _build()
    return _cache["nc"]


def _prep_inputs(embeddings, labels):
    import ml_dtypes

    emb = np.ascontiguousarray(np.asarray(embeddings, dtype=np.float32))
    lab = np.asarray(labels).astype(np.int64).ravel()
    assert emb.shape == (B, D) and lab.shape == (B,)

    counts = np.bincount(lab, minlength=NCLS)
    present = counts[counts > 0]
    # window scheme needs bounded class extent; singleton classes would
    # change validity semantics. Both hold for this problem's data.
    assert present.max() <= MC, f"class too large for window: {present.max()}"
    assert present.min() >= 2, "singleton class unsupported"

    perm = np.argsort(lab, kind="stable")
    emb_s = emb[perm].astype(ml_dtypes.bfloat16)
    lab_s = lab[perm]

    ident = np.eye(128, dtype=ml_dtypes.bfloat16)
    in_maps = []
    for c in range(N_CORES):
        shift = ROLL - BC * c
        emb_l = np.roll(emb_s, shift, axis=0)
        lab_l = np.roll(lab_s, shift)
        htw = (lab_l[None, :HTW_W] == np.arange(NCLS)[:, None]).astype(
            ml_dtypes.bfloat16)
        # pre-tile to partition-major [p, t*128+d] so DMA reads contiguously
        emb_t = emb_l.reshape(64, 128, 128).transpose(1, 0, 2).reshape(128, B)
        htwn = (-8.0 * htw[:, ROLL:ROLL + BC].astype(np.float32)).astype(
            ml_dtypes.bfloat16)
        in_maps.append({
            "emb": np.ascontiguousarray(emb_t),
            "htw": np.ascontiguousarray(htw),
            "htwn": np.ascontiguousarray(htwn),
            "ident": ident,
        })
    return in_maps


def kernel(embeddings, labels, _trace=False):
    from concourse.bass_utils import run_bass_kernel_spmd

    nc = _get_nc()
    in_maps = _prep_inputs(embeddings, labels)
    res = run_bass_kernel_spmd(nc, in_maps, core_ids=list(range(N_CORES)),
                               trace=_trace)
    total = 0.0
    count = 0.0
    for c in range(N_CORES):
        o = np.asarray(res.results[c]["out"], dtype=np.float64)
        total += o[0, 0]
        count += o[0, 1]
    if _trace:
        _cache["last_exec_time_ns"] = res.exec_time_ns
        _cache["last_results"] = res
    return np.float32(total / max(count, 1.0))


# revision 52
# speedup vs baseline: 1.0302x; 1.0138x over previous
"""AdaptiveTripletLoss on 8 TRN2 NeuronCores (Bass/Tile, SPMD).

Math: loss = mean over valid rows of relu(hp - hn + 0.5*(1+hp)) with
  hp = hardest (max) same-label distance, hn = hardest (min) other-label
  distance, distances on L2-normalized embeddings.

Device algorithm (per core, rows sharded):
  G' = En @ En.T - 8 * H @ H.T   (H = one-hot labels, 128 classes = full K)
  Same-label entries of G' sit in [-9,-7], different-label in [-1,1], so
    hn^2 = relu(2 - 2*max_j G')          (max over ALL columns)
    hp^2 = relu(-14 - 2*min_j G')        (min over <=2 static 512-chunks)
  Rows are sorted by label on the host and each core's column copy is
  rotated so its own 1024 rows sit at local columns [512, 1536); the
  same-label columns of a 128-row tile then fall in at most two static
  512-chunks, so the -8*HH' correction and the min-reduce only touch
  those chunks.

  Matmuls run in bf16 (fp32 PSUM accumulation; verified rel err ~1e-5).
  The full-row max scan is split between VectorE (tensor_reduce max) and
  ScalarE (exp + free accum_out => log-sum-exp max, beta=160), keeping
  both engines busy; GpSimd computes the squares for row norms.
"""

import sys

sys.path.insert(0, "/opt/trn_rl_repo")

import numpy as np

N_CORES = 8
B, D, NCLS = 8192, 128, 128
BC = B // N_CORES          # rows per core
ROLL = 512                 # own rows at local columns [ROLL, ROLL+BC)
NQ, QW = 4, 2048           # ET quarters
M_TILES = BC // 128        # 8 row tiles per core
G_GROUPS = 8               # 8 column groups of 1024
HTW_W = 2048               # one-hot window: local chunks 0..3
ACT_GROUPS = (4, 5, 6, 7)  # max-scan groups handled by ScalarE LSE
BETA = 160.0               # LSE sharpness; bias point keeps exp in range
LSE_BIAS = 0.6
MC = 96                    # max class size bound (asserted at prep time)

_cache = {}


def _build():
    import concourse.tile as tile
    from concourse import bacc, mybir

    f32 = mybir.dt.float32
    bf16 = mybir.dt.bfloat16
    AX = mybir.AxisListType
    OP = mybir.AluOpType
    AF = mybir.ActivationFunctionType
    from concourse.bass import MemorySpace

    n_dve = G_GROUPS - len(ACT_GROUPS)
    dve_groups = [g for g in range(G_GROUPS) if g not in ACT_GROUPS]

    nc = bacc.Bacc("TRN2", target_bir_lowering=False, debug=False,
                   num_devices=N_CORES)
    emb_ext = nc.dram_tensor("emb", [128, B], bf16, kind="ExternalInput")
    htw_ext = nc.dram_tensor("htw", [NCLS, HTW_W], bf16, kind="ExternalInput")
    ident_ext = nc.dram_tensor("ident", [128, 128], bf16, kind="ExternalInput")
    out_ext = nc.dram_tensor("out", [1, 2], f32, kind="ExternalOutput")

    with tile.TileContext(nc) as tc:
        with (
            tc.tile_pool(name="persist", bufs=1) as pp,
            tc.tile_pool(name="sq", bufs=2) as sq_pool,
            tc.tile_pool(name="nen", bufs=2) as nen_pool,
            tc.tile_pool(name="hneg", bufs=2) as hneg_pool,
            tc.tile_pool(name="fin", bufs=1) as fin_pool,
            tc.tile_pool(name="tp_ps", bufs=2, space=MemorySpace.PSUM) as tp_ps,
            tc.tile_pool(name="mm_ps", bufs=3, space=MemorySpace.PSUM) as mm_ps,
        ):
            # ---------- persistent SBUF ----------
            enat = [pp.tile([128, QW // 2], bf16, name=f"enat{qh}",
                             tag=f"enat{qh}") for qh in range(NQ * 2)]
            etq = [pp.tile([128, QW], bf16, name=f"et{q}", tag=f"et{q}")
                   for q in range(NQ)]
            htw = pp.tile([NCLS, HTW_W], bf16, name="htw_sb", tag="htw_sb")
            ident = pp.tile([128, 128], bf16, name="ident_sb", tag="ident_sb")
            ssq = pp.tile([128, 64], f32, name="ssq", tag="ssq")
            rvec = pp.tile([128, 64], f32, name="rvec", tag="rvec")
            gmaxf = pp.tile([128, M_TILES * G_GROUPS], f32, name="gmaxf",
                            tag="gmaxf")
            gsum = pp.tile([128, M_TILES * G_GROUPS], f32, name="gsum",
                           tag="gsum")
            gminw = pp.tile([128, M_TILES * 2], f32, name="gminw", tag="gminw")
            bexp = pp.tile([128, 1], f32, name="bexp", tag="bexp")

            nc.gpsimd.memset(bexp[:], -float(BETA * LSE_BIAS))
            nc.gpsimd.memset(gminw[:], 1e9)
            nc.gpsimd.memset(gmaxf[:], -1e9)
            nc.gpsimd.memset(gsum[:], 0.0)
            nc.sync.dma_start(ident[:], ident_ext.ap())
            nc.sync.dma_start(htw[:], htw_ext.ap())

            # ---------- prologue (per quarter, pipelined):
            # load -> ssq -> r -> normalize -> transpose ----------
            emb_ap = emb_ext.ap()
            i32 = mybir.dt.int32
            HW = QW // 2  # half-quarter: 8 row tiles = 1024 rows
            for qh in range(NQ * 2):
                # host pre-tiles emb to [p, t*128+d]; contiguous per partition
                nc.sync.dma_start(
                    enat[qh][:],
                    emb_ap[:, qh * 1024:(qh + 1) * 1024])
            # all hneg tiles upfront (tiny)
            hnegs = []
            for m in range(M_TILES):
                c0 = ROLL + m * 128
                hneg = hneg_pool.tile([128, 128], bf16, tag=f"hneg{m}",
                                      bufs=1)
                nc.vector.tensor_scalar_mul(hneg[:], htw[:, c0:c0 + 128], -8.0)
                hnegs.append(hneg)

            def build_quarter(q):
                for h in range(2):
                    qh = q * 2 + h
                    hs = h * HW
                    sq = sq_pool.tile([128, HW], f32)
                    nc.gpsimd.tensor_tensor(sq[:], enat[qh][:],
                                            enat[qh][:],
                                            op=OP.mult)
                    o = q * 16 + h * 8
                    qs = ssq[:, o:o + 8]
                    nc.vector.tensor_reduce(
                        qs, sq[:].rearrange("p (t d) -> p t d", d=128),
                        axis=AX.X, op=OP.add)
                    # r = rsqrt(ssq) on DVE only: int magic + 2 Newton steps
                    rq = rvec[:, o:o + 8]
                    yi = fin_pool.tile([128, 8], i32, tag="yi", bufs=2)
                    nc.vector.tensor_scalar(yi[:], qs.bitcast(i32), 1, None,
                                            op0=OP.arith_shift_right)
                    nc.vector.tensor_scalar(yi[:], yi[:], -1, 0x5F3759DF,
                                            op0=OP.mult, op1=OP.add)
                    y = yi[:].bitcast(f32)
                    w = fin_pool.tile([128, 8], f32, tag="w", bufs=2)
                    for _ in range(2):
                        nc.vector.tensor_tensor(w[:], y, y, op=OP.mult)
                        nc.vector.tensor_tensor(w[:], w[:], qs, op=OP.mult)
                        nc.vector.tensor_scalar(w[:], w[:], -0.5, 1.5,
                                                op0=OP.mult, op1=OP.add)
                        nc.vector.tensor_tensor(rq, y, w[:], op=OP.mult)
                        y = rq

                    # batched diag build: diag[p, t, j] = ident[p,j]*r[p,t]
                    diag = nen_pool.tile([128, HW], bf16)
                    rb = rq.rearrange("p (t o) -> p t o", o=1) \
                        .broadcast_to((128, 8, 128))
                    ib = ident[:].rearrange("p (o j) -> p o j", o=1) \
                        .broadcast_to((128, 8, 128))
                    nc.vector.tensor_tensor(
                        diag[:].rearrange("p (t d) -> p t d", d=128),
                        ib, rb, op=OP.mult)

                    # normalize+transpose in one matmul per tile:
                    # out[d,j] = sum_k E[k,d]*r_k*delta_kj = E[j,d]*r_j
                    for t4 in range(2):
                        tp = tp_ps.tile([128, 512], f32, tag="tp")
                        for tt in range(4):
                            t = t4 * 4 + tt
                            nc.tensor.matmul(
                                tp[:, tt * 128:(tt + 1) * 128],
                                enat[qh][:, t * 128:(t + 1) * 128],
                                diag[:, t * 128:(t + 1) * 128],
                                start=True, stop=True)
                        nc.scalar.copy(
                            etq[q][:, hs + t4 * 512:hs + (t4 + 1) * 512],
                            tp[:])

            # ---------- main: G' tiles + row reduces (group-outer) ----------
            def main_group(g, m):
                c0 = ROLL + m * 128
                win_chunks = sorted({(c0 - MC + 1) // 512,
                                     (c0 + 127 + MC) // 512})
                kxm_e = etq[0][:, c0:c0 + 128]
                act_ms = (0, 1, 3, 5, 6) if g < 4 else (2, 4, 7)
                use_act = m in act_ms
                ps = mm_ps.tile([128, 1024], f32, tag="mmg", name="ps")
                qq, qoff = g // 2, (g % 2) * 1024
                # E matmuls at PSUM-bank (512) granularity; windowed
                # chunks get the -8*HH' accumulation on top
                for cc in range(2):
                    a, b = cc * 512, (cc + 1) * 512
                    winpart = (2 * g + cc) in win_chunks
                    nc.tensor.matmul(
                        ps[:, a:b], kxm_e,
                        etq[qq][:, qoff + a:qoff + b],
                        start=True, stop=not winpart)
                for cc in range(2):
                    ch = 2 * g + cc
                    if ch in win_chunks:
                        a, b = cc * 512, (cc + 1) * 512
                        nc.tensor.matmul(
                            ps[:, a:b], hnegs[m][:],
                            htw[:, ch * 512:(ch + 1) * 512],
                            start=False, stop=True)
                        # pos side: min of G' over this window chunk
                        k = m * 2 + win_chunks.index(ch)
                        nc.vector.tensor_reduce(
                            gminw[:, k:k + 1],
                            ps[:, a:b], axis=AX.X, op=OP.min)
                # neg side: max of G' over everything; split DVE/ACT so ACT
                # gets more groups early (it idles during the prologue)
                k = m * G_GROUPS + g
                if use_act:
                    nc.scalar.activation(ps[:], ps[:], AF.Exp,
                                         scale=float(BETA), bias=bexp[:],
                                         accum_out=gsum[:, k:k + 1])
                else:
                    nc.vector.tensor_reduce(
                        gmaxf[:, k:k + 1], ps[:], axis=AX.X, op=OP.max)

            for q in range(NQ):
                build_quarter(q)
                for g in (2 * q, 2 * q + 1):
                    for m in range(M_TILES):
                        main_group(g, m)

            # ---------- finalize (all small ops; DVE + bit tricks) ----------
            # direct part of the max scan
            gmaxd = fin_pool.tile([128, M_TILES], f32, tag="gmaxd")
            nc.vector.tensor_reduce(
                gmaxd[:], gmaxf[:].rearrange("p (m g) -> p m g", g=G_GROUPS),
                axis=AX.X, op=OP.max)
            # LSE part: max ~= LSE_BIAS + ln(sum S)/beta, with
            # ln(x) ~= (float(bits(x)) * 2^-23 - 126.9427) * ln2
            ssum = fin_pool.tile([128, M_TILES], f32, tag="ssum")
            nc.vector.tensor_reduce(
                ssum[:], gsum[:].rearrange("p (m g) -> p m g", g=G_GROUPS),
                axis=AX.X, op=OP.add)
            ssumf = fin_pool.tile([128, M_TILES], f32, tag="ssumf")
            nc.vector.tensor_scalar_max(ssumf[:], ssum[:], 1e-37)
            ssbits = fin_pool.tile([128, M_TILES], f32, tag="ssbits")
            nc.vector.tensor_copy(ssbits[:], ssumf[:].bitcast(i32))
            LN2 = 0.69314718056
            lsemax = fin_pool.tile([128, M_TILES], f32, tag="lsemax")
            nc.vector.tensor_scalar(
                lsemax[:], ssbits[:], LN2 / (BETA * 8388608.0),
                LSE_BIAS - 126.94269504 * LN2 / BETA, op0=OP.mult, op1=OP.add)
            gmax8 = fin_pool.tile([128, M_TILES], f32, tag="gmax8")
            nc.vector.tensor_tensor(gmax8[:], gmaxd[:], lsemax[:], op=OP.max)

            gmin8 = fin_pool.tile([128, M_TILES], f32, tag="gmin8")
            nc.vector.tensor_reduce(
                gmin8[:], gminw[:].rearrange("p (m k) -> p m k", k=2),
                axis=AX.X, op=OP.min)

            # hn = sqrt(relu(2-2*gmax8)), hp = sqrt(relu(-14-2*gmin8));
            # sqrt(x) = x * rsqrt(x) with magic + 2 Newton steps (no ACT)
            def dve_sqrt(name, src_ap, scale, bias):
                x = fin_pool.tile([128, M_TILES], f32, tag=f"{name}_x")
                nc.vector.tensor_scalar(x[:], src_ap, scale, bias,
                                        op0=OP.mult, op1=OP.add)
                xm = fin_pool.tile([128, M_TILES], f32, tag=f"{name}_xm")
                nc.vector.tensor_scalar_max(xm[:], x[:], 1e-20)
                yi = fin_pool.tile([128, M_TILES], i32, tag=f"{name}_yi")
                nc.vector.tensor_scalar(yi[:], xm[:].bitcast(i32), 1, None,
                                        op0=OP.arith_shift_right)
                nc.vector.tensor_scalar(yi[:], yi[:], -1, 0x5F3759DF,
                                        op0=OP.mult, op1=OP.add)
                y = yi[:].bitcast(f32)
                w = fin_pool.tile([128, M_TILES], f32, tag=f"{name}_w")
                for _ in range(2):
                    nc.vector.tensor_tensor(w[:], y, y, op=OP.mult)
                    nc.vector.tensor_tensor(w[:], w[:], xm[:], op=OP.mult)
                    nc.vector.tensor_scalar(w[:], w[:], -0.5, 1.5,
                                            op0=OP.mult, op1=OP.add)
                    nc.vector.tensor_tensor(y, y, w[:], op=OP.mult)
                out = fin_pool.tile([128, M_TILES], f32, tag=f"{name}")
                nc.vector.tensor_tensor(out[:], y, xm[:], op=OP.mult)
                return out

            hn = dve_sqrt("hn", gmax8[:], -2.0, 2.0)
            hp = dve_sqrt("hp", gmin8[:], -2.0, -14.0)

            # every row is valid (asserted at prep: all classes >=2 members,
            # >1 class present), so just sum the per-row losses
            t1 = fin_pool.tile([128, M_TILES], f32, tag="t1")
            nc.vector.tensor_scalar(t1[:], hp[:], 1.5, 0.5,
                                    op0=OP.mult, op1=OP.add)
            t2 = fin_pool.tile([128, M_TILES], f32, tag="t2")
            nc.vector.tensor_tensor(t2[:], t1[:], hn[:], op=OP.subtract)
            t3 = fin_pool.tile([128, M_TILES], f32, tag="t3")
            nc.vector.tensor_scalar_max(t3[:], t2[:], 0.0)

            stacked = fin_pool.tile([128, 2], f32, tag="stacked")
            nc.vector.tensor_reduce(stacked[:, 0:1], t3[:], axis=AX.X, op=OP.add)
            nc.gpsimd.memset(stacked[:, 1:2], float(M_TILES))

            ones = fin_pool.tile([128, 1], f32, tag="ones")
            nc.gpsimd.memset(ones[:], 1.0)
            pfin = tp_ps.tile([1, 2], f32, tag="tp")
            nc.tensor.matmul(pfin[:], ones[:], stacked[:], start=True, stop=True)
            outsb = fin_pool.tile([1, 2], f32, tag="outsb")
            nc.vector.tensor_copy(outsb[:], pfin[:])
            nc.sync.dma_start(out_ext.ap(), outsb[:])

    nc.compile()
    return nc


def _get_nc():
    if "nc" not in _cache:
        _cache["nc"] = _build()
    return _cache["nc"]


def _prep_inputs(embeddings, labels):
    import ml_dtypes

    emb = np.ascontiguousarray(np.asarray(embeddings, dtype=np.float32))
    lab = np.asarray(labels).astype(np.int64).ravel()
    assert emb.shape == (B, D) and lab.shape == (B,)

    counts = np.bincount(lab, minlength=NCLS)
    present = counts[counts > 0]
    # window scheme needs bounded class extent; singleton classes would
    # change validity semantics. Both hold for this problem's data.
    assert present.max() <= MC, f"class too large for window: {present.max()}"
    assert present.min() >= 2, "singleton class unsupported"

    perm = np.argsort(lab, kind="stable")
    emb_s = emb[perm].astype(ml_dtypes.bfloat16)
    lab_s = lab[perm]

    ident = np.eye(128, dtype=ml_dtypes.bfloat16)
    in_maps = []
    for c in range(N_CORES):
        shift = ROLL - BC * c
        emb_l = np.roll(emb_s, shift, axis=0)
        lab_l = np.roll(lab_s, shift)
        htw = (lab_l[None, :HTW_W] == np.arange(NCLS)[:, None]).astype(
            ml_dtypes.bfloat16)
        # pre-tile to partition-major [p, t*128+d] so DMA reads contiguously
        emb_t = emb_l.reshape(64, 128, 128).transpose(1, 0, 2).reshape(128, B)
        in_maps.append({
            "emb": np.ascontiguousarray(emb_t),
            "htw": np.ascontiguousarray(htw),
            "ident": ident,
        })
    return in_maps


def kernel(embeddings, labels, _trace=False):
    from concourse.bass_utils import run_bass_kernel_spmd

    nc = _get_nc()
    in_maps = _prep_inputs(embeddings, labels)
    res = run_bass_kernel_spmd(nc, in_maps, core_ids=list(range(N_CORES)),
                               trace=_trace)
    total = 0.0
    count = 0.0
    for c in range(N_CORES):
        o = np.asarray(res.results[c]["out"], dtype=np.float64)
        total += o[0, 0]
        count += o[0, 1]
    if _trace:
        _cache["last_exec_time_ns"] = res.exec_time_ns
        _cache["last_results"] = res
    return np.float32(total / max(count, 1.0))
